# revision 5
# baseline (speedup 1.0000x reference)
"""Multi-resolution 3D conv (3x3x3, Cin=Cout=16) on 8 TRN2 NeuronCores.

Strategy:
- 8 cores = 4 batches x 2 z-halves. Each core runs an INDEPENDENT
  single-device bass exec (no shard_map barrier), so host->device upload,
  compute, and device->host download of different cores pipeline over the
  (slow, ~48MB/s shared) axon tunnel. The tunnel is the bottleneck, so
  everything is about minimizing bytes on the wire.
- All tunnel traffic is fp16 (tolerance 2e-2 >> fp16 error ~5e-4):
  one fused input tensor per core [weights | bias | 4 level slabs], one
  fused output tensor per core. No zero output buffers are uploaded:
  every output element is written by the kernel, so persistent on-device
  dummies stand in for the zero-init operands.
- No spatial zero-padding is uploaded either. Input slabs are exactly
  [R*16, Zi, R] (z-halo only). SAME-conv y-boundary handling is done with
  partial-width matmul taps; x-boundary handling with partition memsets
  on the first/last x-block tiles. Output is exactly [R*16, Zo, R].
- Per level (R in 16/32/48/64): 3x3x3 conv = 9 PSUM-accumulated matmuls,
  one per (kz, ky) tap. The kx 3-tap conv is folded into a block-Toeplitz
  stationary matrix: K = 8 input x-positions x 16 Cin = 128 partitions,
  M = 6 output x-positions x 16 Cout = 96. The last x-block is SHIFTED to
  x = R-6 (overlapping the previous block); only its non-overlapping
  output columns are written to DRAM.
"""

import os
import sys
import threading
from concurrent.futures import ThreadPoolExecutor

for _p in ("/opt/trn_rl_repo",):
    if os.path.isdir(_p) and _p not in sys.path:
        sys.path.insert(0, _p)

import numpy as np

import concourse.bacc as bacc
import concourse.mybir as mybir
from concourse.tile import TileContext

RESOLUTIONS = (16, 32, 48, 64)
B, C = 4, 16
N_TOTAL = sum(r**3 for r in RESOLUTIONS)
XBO, XBI = 6, 8  # output / input x-positions per x-block
F16 = mybir.dt.float16
F32 = mybir.dt.float32

# nz: output z-rows per matmul chunk (nz * R <= 512, one PSUM bank)
NZ = {64: 8, 48: 8, 32: 16, 16: 8}

WT_ELEMS = 128 * 9 * 96
BS_ELEMS = 96

# tap order: first tap is the full-width (b=1) one so its start=True
# write covers every PSUM element of the chunk
TAPS = [(0, 1), (0, 0), (0, 2), (1, 1), (1, 0), (1, 2), (2, 1), (2, 0), (2, 2)]


class _Lvl:
    def __init__(self, R):
        self.R = R
        self.Xp = -(-R // XBO)          # number of x-blocks
        self.Zo = R // 2                # output z-rows per core
        self.Zi = self.Zo + 2           # z rows incl halo
        self.in_elems = R * C * self.Zi * R    # [R*16, Zi, R]
        self.out_elems = R * C * self.Zo * R   # [R*16, Zo, R]
        self.w_last = R - (self.Xp - 1) * XBO  # fresh x-cols in last block

    def xo(self, xb):
        return min(xb * XBO, self.R - XBO)


LVLS = {R: _Lvl(R) for R in RESOLUTIONS}
ORDER = (64, 48, 32, 16)  # big level first keeps PE warm

# fused input layout per core: [wt | bias | lvl64 | lvl48 | lvl32 | lvl16]
XOFF = {}
_o = WT_ELEMS + BS_ELEMS
for _R in ORDER:
    XOFF[_R] = _o
    _o += LVLS[_R].in_elems
XN = _o
YOFF = {}
_o = 0
for _R in ORDER:
    YOFF[_R] = _o
    _o += LVLS[_R].out_elems
YN = _o

LOFF = {}  # level offset in the flat [B, N_TOTAL, C] input
_o = 0
for _R in RESOLUTIONS:
    LOFF[_R] = _o
    _o += _R**3


def _build_nc():
    nc = bacc.Bacc(target_bir_lowering=False)
    xin = nc.dram_tensor("xin", [XN], F16, kind="ExternalInput")
    yout = nc.dram_tensor("yout", [YN], F16, kind="ExternalOutput")

    wt = xin[0:WT_ELEMS].rearrange("(k t m) -> k t m", t=9, m=96)
    bs = xin[WT_ELEMS : WT_ELEMS + BS_ELEMS].rearrange("(p o) -> p o", o=1)
    xs, ys = {}, {}
    for R in ORDER:
        lv = LVLS[R]
        xs[R] = xin[XOFF[R] : XOFF[R] + lv.in_elems].rearrange(
            "(p z y) -> p z y", z=lv.Zi, y=lv.R
        )
        ys[R] = yout[YOFF[R] : YOFF[R] + lv.out_elems].rearrange(
            "(p z y) -> p z y", z=lv.Zo, y=lv.R
        )

    with TileContext(nc) as tc:
        with (
            tc.tile_pool(name="wp", bufs=1) as wp,
            tc.tile_pool(name="ip", bufs=6) as ip,
            tc.tile_pool(name="op", bufs=6) as op,
            tc.tile_pool(name="pp", bufs=4, space="PSUM") as pp,
            tc.tile_pool(name="dp", bufs=1, space="PSUM") as dp,
        ):
            wt_sb = wp.tile([128, 9, 96], F16, name="wt_sb")
            nc.sync.dma_start(wt_sb[:, :, :], wt)
            bs_sb = wp.tile([96, 1], F16, name="bs_sb")
            nc.sync.dma_start(bs_sb[:, :], bs)
            # The S3 LDWEIGHTS slot only fits one semaphore wait, so make
            # sure every real matmul needs at most one: absorb each DMA's
            # completion wait with a tiny throwaway PE/DVE op first.
            dps = dp.tile([1, 2], F32, name="dps")
            scr = wp.tile([96, 1], F32, name="scr")
            nc.tensor.matmul(
                dps[0:1, 0:1], wt_sb[:, 0, 0:1], wt_sb[:, 0, 0:1],
                start=True, stop=True,
            )
            # scr doubles as the fp32 bias used by every tensor_scalar_add
            nc.vector.tensor_copy(scr[:, :], bs_sb[:, :])

            for R in ORDER:
                lv = LVLS[R]
                nz = NZ[R]
                for xb in range(lv.Xp):
                    xo = lv.xo(xb)
                    first = xb == 0
                    last = xb == lv.Xp - 1
                    it = ip.tile(
                        [128, lv.Zi, lv.R], F16, tag="it", name=f"it{R}_{xb}"
                    )
                    if first:
                        # x = -1 does not exist: zero partitions 0:16 (memset
                        # a 32-aligned span; the DMA overwrites 16:32), then
                        # DMA x in [0, 7) into partitions 16:128
                        nc.vector.memset(it[0:32, :, :], 0.0)
                        nc.sync.dma_start(
                            it[16:128, :, :], xs[R][0:112, :, :]
                        )
                    elif last:
                        # x = R does not exist: zero partitions 112:128
                        # (32-aligned memset; DMA overwrites 96:112), then
                        # DMA x in [R-7, R) into partitions 0:112
                        nc.vector.memset(it[96:128, :, :], 0.0)
                        nc.sync.dma_start(
                            it[0:112, :, :],
                            xs[R][(lv.R - 7) * C :, :, :],
                        )
                    else:
                        nc.sync.dma_start(
                            it[:, :, :],
                            xs[R][(xo - 1) * C : (xo - 1) * C + 128, :, :],
                        )
                    nc.tensor.matmul(
                        dps[0:1, 0:1], it[:, 0, 0:1], it[:, 0, 0:1],
                        start=True, stop=True,
                    )
                    for zc in range(0, lv.Zo, nz):
                        ps = pp.tile([96, nz, R], F32, tag="ps", name=f"ps{R}_{xb}_{zc}")
                        for ti, (a, b) in enumerate(TAPS):
                            # SAME-conv y boundary: tap b contributes to
                            # out y in [max(0,1-b), R-b+1) cap [0, R)
                            ylo, yhi = (1, R) if b == 0 else (0, R - 1) if b == 2 else (0, R)
                            nc.tensor.matmul(
                                ps[:, :, ylo:yhi],
                                wt_sb[:, a * 3 + b, :],
                                it[:, zc + a : zc + a + nz, ylo + b - 1 : yhi + b - 1],
                                start=(ti == 0),
                                stop=(ti == 8),
                            )
                        ot = op.tile([96, nz, R], F16, tag="ot", name=f"ot{R}_{xb}_{zc}")
                        nc.vector.tensor_scalar_add(ot[:, :, :], ps[:, :, :], scr[:, :])
                        if last:
                            skip = XBO - lv.w_last
                            nc.sync.dma_start(
                                ys[R][(xo + skip) * C :, zc : zc + nz, :],
                                ot[skip * C :, :, :],
                            )
                        else:
                            nc.sync.dma_start(
                                ys[R][xo * C : xo * C + 96, zc : zc + nz, :],
                                ot[:, :, :],
                            )
    nc.finalize()
    return nc


def _build_wt(weight):
    # weight [Cout, Cin, kz, ky, kx]; WT[xi*16+ci, t, xo*16+co] = w[co,ci,a,b,xi-xo]
    w = np.asarray(weight, np.float32)
    WT = np.zeros((XBI, C, 9, XBO, C), np.float16)
    for t in range(9):
        a, b = t // 3, t % 3
        for xo_ in range(XBO):
            for d in range(3):
                WT[xo_ + d, :, t, xo_, :] = w[:, :, a, b, d].T
    return np.ascontiguousarray(WT.reshape(WT_ELEMS))


_ST = None  # lazy global state
_ST_LOCK = threading.Lock()


class _State:
    def __init__(self):
        import jax
        from concourse import bass2jax as b2j

        self.jax = jax
        nc = _build_nc()
        b2j.install_neuronx_cc_hook()
        # the kernel body never reads the partition id; bind it as 0
        part_name = (
            nc.partition_id_tensor.name if nc.partition_id_tensor is not None else None
        )
        in_names = ("xin", "yout") + ((part_name,) if part_name else ())

        out_aval = jax.core.ShapedArray((YN,), np.float16)

        def _body(xin_arr, ydummy):
            operands = [xin_arr, ydummy]
            if part_name is not None:
                operands.append(b2j.partition_id_tensor())
            outs = b2j._bass_exec_p.bind(
                *operands,
                out_avals=(out_aval,),
                in_names=in_names,
                out_names=("yout",),
                lowering_input_output_aliases=(),
                sim_require_finite=True,
                sim_require_nnan=True,
                nc=nc,
            )
            return outs[0]

        self.jfn = jax.jit(_body, keep_unused=True)
        self.devs = jax.devices()[:8]

        # persistent on-device stand-ins for the zero-init output operand
        # (every output element is DMA-written by the kernel, so their
        # contents never reach the result)
        mk = jax.jit(lambda a: jax.numpy.broadcast_to(a, (YN,)))
        self.dummies = []
        for d in self.devs:
            anchor = jax.device_put(np.zeros((), np.float16), d)
            self.dummies.append(jax.block_until_ready(mk(anchor)))

        # host buffers: fused per-core input rows
        self.XG = np.zeros((8, XN), np.float16)
        self.fetch_pool = ThreadPoolExecutor(8)
        self.exec_pool = ThreadPoolExecutor(8)


def _get_state():
    global _ST
    if _ST is None:
        with _ST_LOCK:
            if _ST is None:
                _ST = _State()
    return _ST


def _pack_core(st, core, inp):
    bi, h = core // 2, core % 2
    row = st.XG[core]
    for R in RESOLUTIONS:
        lv = LVLS[R]
        x = inp[bi, LOFF[R] : LOFF[R] + R**3].reshape(R, R, R, C)
        zlo = h * lv.Zo - 1
        s0, s1 = max(zlo, 0), min(zlo + lv.Zi, R)
        dst = row[XOFF[R] : XOFF[R] + lv.in_elems].reshape(R, C, lv.Zi, R)
        # z-halo rows outside the grid are zero
        if s0 - zlo > 0:
            dst[:, :, 0 : s0 - zlo, :] = 0
        if zlo + lv.Zi > s1:
            dst[:, :, s1 - zlo :, :] = 0
        np.copyto(dst[:, :, s0 - zlo : s1 - zlo, :], x[s0:s1].transpose(2, 3, 0, 1))


def _unpack_core(core, ya, out):
    bi, h = core // 2, core % 2
    for R in RESOLUTIONS:
        lv = LVLS[R]
        rows = lv.Zo * R * R
        dst = out[
            bi, LOFF[R] + h * rows : LOFF[R] + (h + 1) * rows
        ].reshape(lv.Zo, R, R, C)
        src = ya[YOFF[R] : YOFF[R] + lv.out_elems].reshape(R, C, lv.Zo, R)
        np.copyto(dst, src.transpose(2, 3, 0, 1))


def _run(inputs, trace=False):
    st = _get_state()
    jax = st.jax
    inp = np.asarray(inputs["input"], np.float32)
    weight = np.asarray(inputs["weight"], np.float32)
    bias = np.asarray(inputs["bias"], np.float32)

    wt_row = _build_wt(weight)
    b96 = np.tile(bias.astype(np.float16), XBO)
    for core in range(8):
        st.XG[core, 0:WT_ELEMS] = wt_row
        st.XG[core, WT_ELEMS : WT_ELEMS + BS_ELEMS] = b96

    out = np.empty((B, N_TOTAL, C), np.float32)

    def _exec_and_fetch(core, xdev):
        ydev = st.jfn(xdev, st.dummies[core])
        ya = np.asarray(ydev)
        _unpack_core(core, ya, out)

    futs = []
    for core in range(8):
        _pack_core(st, core, inp)
        xdev = jax.device_put(st.XG[core], st.devs[core])
        futs.append(st.exec_pool.submit(_exec_and_fetch, core, xdev))
    for f in futs:
        f.result()
    return out, None


def kernel(**inputs):
    out, _ = _run(inputs)
    return out


# revision 14
# speedup vs baseline: 1.2619x; 1.2619x over previous
"""Multi-resolution 3D conv (3x3x3, Cin=Cout=16) on 8 TRN2 NeuronCores.

Strategy:
- 8 cores = 4 batches x 2 z-halves. Each core runs an INDEPENDENT
  single-device bass exec (no shard_map barrier), so host->device upload,
  compute, and device->host download of different cores pipeline over the
  (slow, ~48MB/s shared) axon tunnel. The tunnel is the bottleneck, so
  everything is about minimizing bytes on the wire.
- All tunnel traffic is fp16 (tolerance 2e-2 >> fp16 error ~5e-4):
  one fused input tensor per core [weights | bias | 4 level slabs], one
  fused output tensor per core. No zero output buffers are uploaded:
  every output element is written by the kernel, so persistent on-device
  dummies stand in for the zero-init operands.
- No spatial zero-padding is uploaded either. Input slabs are exactly
  [R*16, Zi, R] (z-halo only). SAME-conv y-boundary handling is done with
  partial-width matmul taps; x-boundary handling with partition memsets
  on the first/last x-block tiles. Output is exactly [R*16, Zo, R].
- Per level (R in 16/32/48/64): 3x3x3 conv = 9 PSUM-accumulated matmuls,
  one per (kz, ky) tap. The kx 3-tap conv is folded into a block-Toeplitz
  stationary matrix: K = 8 input x-positions x 16 Cin = 128 partitions,
  M = 6 output x-positions x 16 Cout = 96. The last x-block is SHIFTED to
  x = R-6 (overlapping the previous block); only its non-overlapping
  output columns are written to DRAM.
"""

import os
import sys
import threading
from concurrent.futures import ThreadPoolExecutor

for _p in ("/opt/trn_rl_repo",):
    if os.path.isdir(_p) and _p not in sys.path:
        sys.path.insert(0, _p)

import numpy as np

import concourse.bacc as bacc
import concourse.mybir as mybir
from concourse.tile import TileContext

RESOLUTIONS = (16, 32, 48, 64)
B, C = 4, 16
N_TOTAL = sum(r**3 for r in RESOLUTIONS)
XBO, XBI = 6, 8  # output / input x-positions per x-block
F16 = mybir.dt.float16
F32 = mybir.dt.float32

# nz: output z-rows per matmul chunk (nz * R <= 512, one PSUM bank)
NZ = {64: 8, 48: 8, 32: 16, 16: 8}

WT_ELEMS = 128 * 9 * 96
BS_ELEMS = 96

# tap order: first tap is the full-width (b=1) one so its start=True
# write covers every PSUM element of the chunk
TAPS = [(0, 1), (0, 0), (0, 2), (1, 1), (1, 0), (1, 2), (2, 1), (2, 0), (2, 2)]


class _Lvl:
    def __init__(self, R):
        self.R = R
        self.Xp = -(-R // XBO)          # number of x-blocks
        self.Zo = R // 2                # output z-rows per core
        self.Zi = self.Zo + 2           # z rows incl halo
        self.L = self.Zi * R            # free elems per partition
        self.G = self.L // 4            # 12-bit pack groups per partition
        self.W = 3 * self.G             # packed u16 words per partition
        self.in_elems = R * C * self.W         # packed [R*16, W]
        self.out_elems = R * C * self.Zo * R   # [R*16, Zo, R]
        self.w_last = R - (self.Xp - 1) * XBO  # fresh x-cols in last block

    def xo(self, xb):
        return min(xb * XBO, self.R - XBO)


LVLS = {R: _Lvl(R) for R in RESOLUTIONS}
ORDER = (64, 48, 32, 16)  # big level first keeps PE warm

# fused input layout per core: [wt | bias | lvl64 | lvl48 | lvl32 | lvl16]
XOFF = {}
_o = WT_ELEMS + BS_ELEMS
for _R in ORDER:
    XOFF[_R] = _o
    _o += LVLS[_R].in_elems
XN = _o
YOFF = {}
_o = 0
for _R in ORDER:
    YOFF[_R] = _o
    _o += LVLS[_R].out_elems
YN = _o

LOFF = {}  # level offset in the flat [B, N_TOTAL, C] input
_o = 0
for _R in RESOLUTIONS:
    LOFF[_R] = _o
    _o += _R**3


def _stt_u16(nc, out, in0, imm, in1, op0, op1):
    """scalar_tensor_tensor with a uint16-typed immediate:
    out = (in0 op0 imm) op1 in1. The stock helper types immediates as
    float32, which the BIR verifier rejects for bitvec ops on u16."""
    eng = nc.vector
    return eng.add_instruction(
        mybir.InstTensorScalarPtr(
            name=eng.bass.get_next_instruction_name(),
            is_scalar_tensor_tensor=True,
            op0=op0,
            op1=op1,
            ins=[
                eng.lower_ap(in0),
                mybir.ImmediateValue(dtype=mybir.dt.uint16, value=imm),
                eng.lower_ap(in1),
            ],
            outs=[eng.lower_ap(out)],
        )
    )


def _build_nc():
    nc = bacc.Bacc(target_bir_lowering=False)
    xin = nc.dram_tensor("xin", [XN], F16, kind="ExternalInput")
    yout = nc.dram_tensor("yout", [YN], F16, kind="ExternalOutput")

    wt = xin[0:WT_ELEMS].rearrange("(k t m) -> k t m", t=9, m=96)
    bs = xin[WT_ELEMS : WT_ELEMS + BS_ELEMS].rearrange("(p o) -> p o", o=1)
    xs, ys = {}, {}
    for R in ORDER:
        lv = LVLS[R]
        # packed 12-bit input: [R*16 partitions, W u16 words]
        xs[R] = xin[XOFF[R] : XOFF[R] + lv.in_elems].rearrange(
            "(p w) -> p w", w=lv.W
        )
        ys[R] = yout[YOFF[R] : YOFF[R] + lv.out_elems].rearrange(
            "(p z y) -> p z y", z=lv.Zo, y=lv.R
        )

    U16 = mybir.dt.uint16
    ALU = mybir.AluOpType

    with TileContext(nc) as tc:
        with (
            tc.tile_pool(name="wp", bufs=1) as wp,
            tc.tile_pool(name="qp", bufs=4) as qp,
            tc.tile_pool(name="tp", bufs=4) as tp,
            tc.tile_pool(name="ip", bufs=4) as ip,
            tc.tile_pool(name="op", bufs=6) as op,
            tc.tile_pool(name="pp", bufs=4, space="PSUM") as pp,
            tc.tile_pool(name="dp", bufs=1, space="PSUM") as dp,
        ):
            wt_sb = wp.tile([128, 9, 96], F16, name="wt_sb")
            nc.sync.dma_start(wt_sb[:, :, :], wt)
            bs_sb = wp.tile([96, 1], F16, name="bs_sb")
            nc.sync.dma_start(bs_sb[:, :], bs)
            # The S3 LDWEIGHTS slot only fits one semaphore wait, so make
            # sure every real matmul needs at most one: absorb each DMA's
            # completion wait with a tiny throwaway PE/DVE op first.
            dps = dp.tile([1, 2], F32, name="dps")
            scr = wp.tile([96, 1], F32, name="scr")
            nc.tensor.matmul(
                dps[0:1, 0:1], wt_sb[:, 0, 0:1], wt_sb[:, 0, 0:1],
                start=True, stop=True,
            )
            # scr doubles as the fp32 bias used by every tensor_scalar_add
            nc.vector.tensor_copy(scr[:, :], bs_sb[:, :])

            for R in ORDER:
                lv = LVLS[R]
                nz = NZ[R]
                for xb in range(lv.Xp):
                    xo = lv.xo(xb)
                    first = xb == 0
                    last = xb == lv.Xp - 1
                    # packed 12-bit words for this block's 128 partitions
                    pt = qp.tile([128, lv.W], F16, tag="pt", name=f"pt{R}_{xb}")
                    if first:
                        # x = -1 does not exist: zero words unpack to zero.
                        # memset a 32-aligned span; DMA overwrites 16:32.
                        nc.vector.memset(pt[0:32, :], 0.0)
                        nc.sync.dma_start(pt[16:128, :], xs[R][0:112, :])
                    elif last:
                        nc.vector.memset(pt[96:128, :], 0.0)
                        nc.sync.dma_start(
                            pt[0:112, :], xs[R][(lv.R - 7) * C :, :]
                        )
                    else:
                        nc.sync.dma_start(
                            pt[:, :],
                            xs[R][(xo - 1) * C : (xo - 1) * C + 128, :],
                        )
                    it = ip.tile(
                        [128, lv.Zi, lv.R], F16, tag="it", name=f"it{R}_{xb}"
                    )
                    # DVE unpack: 3 words -> 4 fp16 values per group
                    w3 = pt[:, :].bitcast(U16).rearrange(
                        "p (g t) -> p g t", t=3
                    )
                    o4 = it[:, :, :].bitcast(U16).rearrange(
                        "p z y -> p (z y)"
                    ).rearrange("p (g f) -> p g f", f=4)
                    w0, w1, w2 = w3[:, :, 0], w3[:, :, 1], w3[:, :, 2]
                    tb = tp.tile([128, lv.G], U16, tag="tb", name=f"tb{R}_{xb}")
                    nc.vector.tensor_scalar(
                        o4[:, :, 0], w0, 0xFFF0, None, ALU.bitwise_and
                    )
                    nc.vector.tensor_scalar(
                        tb[:, :], w1, 4, 0x0FF0,
                        ALU.logical_shift_right, ALU.bitwise_and,
                    )
                    _stt_u16(
                        nc, o4[:, :, 1], w0, 12, tb[:, :],
                        ALU.logical_shift_left, ALU.bitwise_or,
                    )
                    nc.vector.tensor_scalar(
                        tb[:, :], w2, 8, 0x00F0,
                        ALU.logical_shift_right, ALU.bitwise_and,
                    )
                    _stt_u16(
                        nc, o4[:, :, 2], w1, 8, tb[:, :],
                        ALU.logical_shift_left, ALU.bitwise_or,
                    )
                    nc.vector.tensor_scalar(
                        o4[:, :, 3], w2, 4, None, ALU.logical_shift_left
                    )
                    nc.tensor.matmul(
                        dps[0:1, 0:1], it[:, 0, 0:1], it[:, 0, 0:1],
                        start=True, stop=True,
                    )
                    for zc in range(0, lv.Zo, nz):
                        ps = pp.tile([96, nz, R], F32, tag="ps", name=f"ps{R}_{xb}_{zc}")
                        for ti, (a, b) in enumerate(TAPS):
                            # SAME-conv y boundary: tap b contributes to
                            # out y in [max(0,1-b), R-b+1) cap [0, R)
                            ylo, yhi = (1, R) if b == 0 else (0, R - 1) if b == 2 else (0, R)
                            nc.tensor.matmul(
                                ps[:, :, ylo:yhi],
                                wt_sb[:, a * 3 + b, :],
                                it[:, zc + a : zc + a + nz, ylo + b - 1 : yhi + b - 1],
                                start=(ti == 0),
                                stop=(ti == 8),
                            )
                        ot = op.tile([96, nz, R], F16, tag="ot", name=f"ot{R}_{xb}_{zc}")
                        nc.vector.tensor_scalar_add(ot[:, :, :], ps[:, :, :], scr[:, :])
                        if last:
                            skip = XBO - lv.w_last
                            nc.sync.dma_start(
                                ys[R][(xo + skip) * C :, zc : zc + nz, :],
                                ot[skip * C :, :, :],
                            )
                        else:
                            nc.sync.dma_start(
                                ys[R][xo * C : xo * C + 96, zc : zc + nz, :],
                                ot[:, :, :],
                            )
    nc.finalize()
    return nc


def _build_wt(weight):
    # weight [Cout, Cin, kz, ky, kx]; WT[xi*16+ci, t, xo*16+co] = w[co,ci,a,b,xi-xo]
    w = np.asarray(weight, np.float32)
    WT = np.zeros((XBI, C, 9, XBO, C), np.float16)
    for t in range(9):
        a, b = t // 3, t % 3
        for xo_ in range(XBO):
            for d in range(3):
                WT[xo_ + d, :, t, xo_, :] = w[:, :, a, b, d].T
    return np.ascontiguousarray(WT.reshape(WT_ELEMS))


_ST = None  # lazy global state
_ST_LOCK = threading.Lock()


class _State:
    def __init__(self):
        import jax
        from concourse import bass2jax as b2j

        self.jax = jax
        nc = _build_nc()
        b2j.install_neuronx_cc_hook()
        # the kernel body never reads the partition id; bind it as 0
        part_name = (
            nc.partition_id_tensor.name if nc.partition_id_tensor is not None else None
        )
        in_names = ("xin", "yout") + ((part_name,) if part_name else ())

        out_aval = jax.core.ShapedArray((YN,), np.float16)

        def _body(xin_arr, ydummy):
            operands = [xin_arr, ydummy]
            if part_name is not None:
                operands.append(b2j.partition_id_tensor())
            outs = b2j._bass_exec_p.bind(
                *operands,
                out_avals=(out_aval,),
                in_names=in_names,
                out_names=("yout",),
                lowering_input_output_aliases=(),
                sim_require_finite=True,
                sim_require_nnan=True,
                nc=nc,
            )
            return outs[0]

        self.jfn = jax.jit(_body, keep_unused=True)
        self.devs = jax.devices()[:8]

        # persistent on-device stand-ins for the zero-init output operand
        # (every output element is DMA-written by the kernel, so their
        # contents never reach the result)
        mk = jax.jit(lambda a: jax.numpy.broadcast_to(a, (YN,)))
        self.dummies = []
        for d in self.devs:
            anchor = jax.device_put(np.zeros((), np.float16), d)
            self.dummies.append(jax.block_until_ready(mk(anchor)))

        # host buffers: fused per-core input rows + transposed fp16 scratch
        self.XG = np.zeros((8, XN), np.float16)
        self.S = {}
        for core in range(8):
            for R in RESOLUTIONS:
                lv = LVLS[R]
                self.S[(core, R)] = np.zeros((R, C, lv.Zi, R), np.float16)
        self.fetch_pool = ThreadPoolExecutor(8)
        self.exec_pool = ThreadPoolExecutor(8)


def _get_state():
    global _ST
    if _ST is None:
        with _ST_LOCK:
            if _ST is None:
                _ST = _State()
    return _ST


def _pack_core(st, core, inp):
    bi, h = core // 2, core % 2
    row = st.XG[core]
    for R in RESOLUTIONS:
        lv = LVLS[R]
        x = inp[bi, LOFF[R] : LOFF[R] + R**3].reshape(R, R, R, C)
        zlo = h * lv.Zo - 1
        s0, s1 = max(zlo, 0), min(zlo + lv.Zi, R)
        S = st.S[(core, R)]
        # z-halo rows outside the grid are zero (S is zero-initialized and
        # the zero rows are per-core constant, but keep it explicit + cheap)
        np.copyto(S[:, :, s0 - zlo : s1 - zlo, :], x[s0:s1].transpose(2, 3, 0, 1))
        # pack fp16 -> 12 bit (round-to-nearest via +8 on the u16 view;
        # safe: no inf/nan and |x| << fp16 max). 4 values -> 3 words.
        A = S.reshape(R * C, lv.L).view(np.uint16) + np.uint16(8)
        a0, a1, a2, a3 = A[:, 0::4], A[:, 1::4], A[:, 2::4], A[:, 3::4]
        Wd = row[XOFF[R] : XOFF[R] + lv.in_elems].view(np.uint16).reshape(
            R * C, lv.G, 3
        )
        np.bitwise_or(a0 & np.uint16(0xFFF0), a1 >> 12, out=Wd[:, :, 0])
        np.bitwise_or((a1 << 4) & np.uint16(0xFF00), a2 >> 8, out=Wd[:, :, 1])
        np.bitwise_or((a2 << 8) & np.uint16(0xF000), a3 >> 4, out=Wd[:, :, 2])


def _unpack_core(core, ya, out):
    bi, h = core // 2, core % 2
    for R in RESOLUTIONS:
        lv = LVLS[R]
        rows = lv.Zo * R * R
        dst = out[
            bi, LOFF[R] + h * rows : LOFF[R] + (h + 1) * rows
        ].reshape(lv.Zo, R, R, C)
        src = ya[YOFF[R] : YOFF[R] + lv.out_elems].reshape(R, C, lv.Zo, R)
        np.copyto(dst, src.transpose(2, 3, 0, 1))


def _run(inputs, trace=False):
    st = _get_state()
    jax = st.jax
    inp = np.asarray(inputs["input"], np.float32)
    weight = np.asarray(inputs["weight"], np.float32)
    bias = np.asarray(inputs["bias"], np.float32)

    wt_row = _build_wt(weight)
    b96 = np.tile(bias.astype(np.float16), XBO)
    for core in range(8):
        st.XG[core, 0:WT_ELEMS] = wt_row
        st.XG[core, WT_ELEMS : WT_ELEMS + BS_ELEMS] = b96

    out = np.empty((B, N_TOTAL, C), np.float32)

    def _exec_and_fetch(core, xdev):
        ydev = st.jfn(xdev, st.dummies[core])
        ya = np.asarray(ydev)
        _unpack_core(core, ya, out)

    futs = []
    for core in range(8):
        _pack_core(st, core, inp)
        xdev = jax.device_put(st.XG[core], st.devs[core])
        futs.append(st.exec_pool.submit(_exec_and_fetch, core, xdev))
    for f in futs:
        f.result()
    return out, None


def kernel(**inputs):
    out, _ = _run(inputs)
    return out


# revision 21
# speedup vs baseline: 1.4834x; 1.1755x over previous
"""Multi-resolution 3D conv (3x3x3, Cin=Cout=16) on 8 TRN2 NeuronCores.

Strategy:
- 8 cores = 4 batches x 2 z-halves. Each core runs an INDEPENDENT
  single-device bass exec (no shard_map barrier), so host->device upload,
  compute, and device->host download of different cores pipeline over the
  (slow, ~48MB/s shared) axon tunnel. The tunnel is the bottleneck, so
  everything is about minimizing bytes on the wire.
- All tunnel traffic is fp16 (tolerance 2e-2 >> fp16 error ~5e-4):
  one fused input tensor per core [weights | bias | 4 level slabs], one
  fused output tensor per core. No zero output buffers are uploaded:
  every output element is written by the kernel, so persistent on-device
  dummies stand in for the zero-init operands.
- No spatial zero-padding is uploaded either. Input slabs are exactly
  [R*16, Zi, R] (z-halo only). SAME-conv y-boundary handling is done with
  partial-width matmul taps; x-boundary handling with partition memsets
  on the first/last x-block tiles. Output is exactly [R*16, Zo, R].
- Per level (R in 16/32/48/64): 3x3x3 conv = 9 PSUM-accumulated matmuls,
  one per (kz, ky) tap. The kx 3-tap conv is folded into a block-Toeplitz
  stationary matrix: K = 8 input x-positions x 16 Cin = 128 partitions,
  M = 6 output x-positions x 16 Cout = 96. The last x-block is SHIFTED to
  x = R-6 (overlapping the previous block); only its non-overlapping
  output columns are written to DRAM.
"""

import os
import sys
import threading
from concurrent.futures import ThreadPoolExecutor

for _p in ("/opt/trn_rl_repo",):
    if os.path.isdir(_p) and _p not in sys.path:
        sys.path.insert(0, _p)

import numpy as np

import concourse.bacc as bacc
import concourse.mybir as mybir
from concourse.tile import TileContext

RESOLUTIONS = (16, 32, 48, 64)
B, C = 4, 16
N_TOTAL = sum(r**3 for r in RESOLUTIONS)
XBO, XBI = 6, 8  # output / input x-positions per x-block
F16 = mybir.dt.float16
F32 = mybir.dt.float32

# nz: output z-rows per matmul chunk (nz * R <= 512, one PSUM bank)
NZ = {64: 8, 48: 8, 32: 16, 16: 8}

WT_ELEMS = 128 * 9 * 96
BS_ELEMS = 96

# tap order: first tap is the full-width (b=1) one so its start=True
# write covers every PSUM element of the chunk
TAPS = [(0, 1), (0, 0), (0, 2), (1, 1), (1, 0), (1, 2), (2, 1), (2, 0), (2, 2)]


class _Lvl:
    def __init__(self, R):
        self.R = R
        self.Xp = -(-R // XBO)          # number of x-blocks
        self.Zo = R // 2                # output z-rows per core
        self.Zi = self.Zo + 2           # z rows incl halo
        self.L = self.Zi * R            # free elems per partition
        self.G = self.L // 4            # 12-bit pack groups per partition
        self.W = 3 * self.G             # packed u16 words per partition
        self.in_elems = R * C * self.W         # packed [R*16, W]
        nz = NZ[R]
        self.L2 = nz * R                # out elems per partition per chunk
        self.G2 = self.L2 // 4
        self.W2 = 3 * self.G2
        self.nch = self.Zo // nz
        self.out_elems = R * C * self.nch * self.W2  # packed [R*16, nch, W2]
        self.w_last = R - (self.Xp - 1) * XBO  # fresh x-cols in last block

    def xo(self, xb):
        return min(xb * XBO, self.R - XBO)


LVLS = {R: _Lvl(R) for R in RESOLUTIONS}
ORDER = (64, 48, 32, 16)  # big level first keeps PE warm

# fused input layout per core: [wt | bias | lvl64 | lvl48 | lvl32 | lvl16]
XOFF = {}
_o = WT_ELEMS + BS_ELEMS
for _R in ORDER:
    XOFF[_R] = _o
    _o += LVLS[_R].in_elems
XN = _o
YOFF = {}
_o = 0
for _R in ORDER:
    YOFF[_R] = _o
    _o += LVLS[_R].out_elems
YN = _o

LOFF = {}  # level offset in the flat [B, N_TOTAL, C] input
_o = 0
for _R in RESOLUTIONS:
    LOFF[_R] = _o
    _o += _R**3


def _stt_u16(nc, out, in0, imm, in1, op0, op1):
    """scalar_tensor_tensor with a uint16-typed immediate:
    out = (in0 op0 imm) op1 in1. The stock helper types immediates as
    float32, which the BIR verifier rejects for bitvec ops on u16."""
    eng = nc.vector
    return eng.add_instruction(
        mybir.InstTensorScalarPtr(
            name=eng.bass.get_next_instruction_name(),
            is_scalar_tensor_tensor=True,
            op0=op0,
            op1=op1,
            ins=[
                eng.lower_ap(in0),
                mybir.ImmediateValue(dtype=mybir.dt.uint16, value=imm),
                eng.lower_ap(in1),
            ],
            outs=[eng.lower_ap(out)],
        )
    )


def _build_nc():
    nc = bacc.Bacc(target_bir_lowering=False)
    xin = nc.dram_tensor("xin", [XN], F16, kind="ExternalInput")
    yout = nc.dram_tensor("yout", [YN], F16, kind="ExternalOutput")

    wt = xin[0:WT_ELEMS].rearrange("(k t m) -> k t m", t=9, m=96)
    bs = xin[WT_ELEMS : WT_ELEMS + BS_ELEMS].rearrange("(p o) -> p o", o=1)
    xs, ys = {}, {}
    for R in ORDER:
        lv = LVLS[R]
        # packed 12-bit input: [R*16 partitions, W u16 words]
        xs[R] = xin[XOFF[R] : XOFF[R] + lv.in_elems].rearrange(
            "(p w) -> p w", w=lv.W
        )
        # packed 12-bit output: [R*16 partitions, z-chunks, W2 words]
        ys[R] = yout[YOFF[R] : YOFF[R] + lv.out_elems].rearrange(
            "(p c w) -> p c w", c=lv.nch, w=lv.W2
        )

    U16 = mybir.dt.uint16
    ALU = mybir.AluOpType

    with TileContext(nc) as tc:
        with (
            tc.tile_pool(name="wp", bufs=1) as wp,
            tc.tile_pool(name="qp", bufs=4) as qp,
            tc.tile_pool(name="tp", bufs=4) as tp,
            tc.tile_pool(name="ip", bufs=4) as ip,
            tc.tile_pool(name="op", bufs=6) as op,
            tc.tile_pool(name="pp", bufs=4, space="PSUM") as pp,
            tc.tile_pool(name="dp", bufs=1, space="PSUM") as dp,
        ):
            wt_sb = wp.tile([128, 9, 96], F16, name="wt_sb")
            nc.sync.dma_start(wt_sb[:, :, :], wt)
            bs_sb = wp.tile([96, 1], F16, name="bs_sb")
            nc.sync.dma_start(bs_sb[:, :], bs)
            # The S3 LDWEIGHTS slot only fits one semaphore wait, so make
            # sure every real matmul needs at most one: absorb each DMA's
            # completion wait with a tiny throwaway PE/DVE op first.
            dps = dp.tile([1, 2], F32, name="dps")
            scr = wp.tile([96, 1], F32, name="scr")
            nc.tensor.matmul(
                dps[0:1, 0:1], wt_sb[:, 0, 0:1], wt_sb[:, 0, 0:1],
                start=True, stop=True,
            )
            # scr doubles as the fp32 bias used by every tensor_scalar_add
            nc.vector.tensor_copy(scr[:, :], bs_sb[:, :])

            for R in ORDER:
                lv = LVLS[R]
                nz = NZ[R]
                for xb in range(lv.Xp):
                    xo = lv.xo(xb)
                    first = xb == 0
                    last = xb == lv.Xp - 1
                    # packed 12-bit words for this block's 128 partitions
                    pt = qp.tile([128, lv.W], F16, tag="pt", name=f"pt{R}_{xb}")
                    if first:
                        # x = -1 does not exist: zero words unpack to zero.
                        # memset a 32-aligned span; DMA overwrites 16:32.
                        nc.vector.memset(pt[0:32, :], 0.0)
                        nc.sync.dma_start(pt[16:128, :], xs[R][0:112, :])
                    elif last:
                        nc.vector.memset(pt[96:128, :], 0.0)
                        nc.sync.dma_start(
                            pt[0:112, :], xs[R][(lv.R - 7) * C :, :]
                        )
                    else:
                        nc.sync.dma_start(
                            pt[:, :],
                            xs[R][(xo - 1) * C : (xo - 1) * C + 128, :],
                        )
                    it = ip.tile(
                        [128, lv.Zi, lv.R], F16, tag="it", name=f"it{R}_{xb}"
                    )
                    # DVE unpack: 3 words -> 4 fp16 values per group
                    w3 = pt[:, :].bitcast(U16).rearrange(
                        "p (g t) -> p g t", t=3
                    )
                    o4 = it[:, :, :].bitcast(U16).rearrange(
                        "p z y -> p (z y)"
                    ).rearrange("p (g f) -> p g f", f=4)
                    w0, w1, w2 = w3[:, :, 0], w3[:, :, 1], w3[:, :, 2]
                    tb = tp.tile([128, lv.G], U16, tag="tb", name=f"tb{R}_{xb}")
                    nc.vector.tensor_scalar(
                        o4[:, :, 0], w0, 0xFFF0, None, ALU.bitwise_and
                    )
                    nc.vector.tensor_scalar(
                        tb[:, :], w1, 4, 0x0FF0,
                        ALU.logical_shift_right, ALU.bitwise_and,
                    )
                    _stt_u16(
                        nc, o4[:, :, 1], w0, 12, tb[:, :],
                        ALU.logical_shift_left, ALU.bitwise_or,
                    )
                    nc.vector.tensor_scalar(
                        tb[:, :], w2, 8, 0x00F0,
                        ALU.logical_shift_right, ALU.bitwise_and,
                    )
                    _stt_u16(
                        nc, o4[:, :, 2], w1, 8, tb[:, :],
                        ALU.logical_shift_left, ALU.bitwise_or,
                    )
                    nc.vector.tensor_scalar(
                        o4[:, :, 3], w2, 4, None, ALU.logical_shift_left
                    )
                    nc.tensor.matmul(
                        dps[0:1, 0:1], it[:, 0, 0:1], it[:, 0, 0:1],
                        start=True, stop=True,
                    )
                    for zc in range(0, lv.Zo, nz):
                        ps = pp.tile([96, nz, R], F32, tag="ps", name=f"ps{R}_{xb}_{zc}")
                        for ti, (a, b) in enumerate(TAPS):
                            # SAME-conv y boundary: tap b contributes to
                            # out y in [max(0,1-b), R-b+1) cap [0, R)
                            ylo, yhi = (1, R) if b == 0 else (0, R - 1) if b == 2 else (0, R)
                            nc.tensor.matmul(
                                ps[:, :, ylo:yhi],
                                wt_sb[:, a * 3 + b, :],
                                it[:, zc + a : zc + a + nz, ylo + b - 1 : yhi + b - 1],
                                start=(ti == 0),
                                stop=(ti == 8),
                            )
                        ot = op.tile([96, nz, R], F16, tag="ot", name=f"ot{R}_{xb}_{zc}")
                        nc.vector.tensor_scalar_add(ot[:, :, :], ps[:, :, :], scr[:, :])
                        # DVE pack fp16 -> 12 bit (round-to-nearest via +8;
                        # the add must be its own instr: no arith+bitwise mix)
                        au = tp.tile([96, lv.L2], U16, tag="au", name=f"au{R}_{xb}_{zc}")
                        nc.vector.tensor_scalar(
                            au[:, :],
                            ot[:, :, :].bitcast(U16).rearrange("p z y -> p (z y)"),
                            8, None, ALU.add,
                        )
                        a4 = au[:, :].rearrange("p (g f) -> p g f", f=4)
                        a0, a1, a2, a3 = (a4[:, :, j] for j in range(4))
                        pw = op.tile([96, lv.W2], F16, tag="pw", name=f"pw{R}_{xb}_{zc}")
                        w3o = pw[:, :].bitcast(U16).rearrange(
                            "p (g t) -> p g t", t=3
                        )
                        T1 = tp.tile([96, lv.G2], U16, tag="T1", name=f"T1{R}_{xb}_{zc}")
                        T2 = tp.tile([96, lv.G2], U16, tag="T2", name=f"T2{R}_{xb}_{zc}")
                        tm = tp.tile([96, lv.G2], U16, tag="tm", name=f"tm{R}_{xb}_{zc}")
                        nc.vector.tensor_scalar(
                            T1[:, :], a1, 4, None, ALU.logical_shift_right
                        )
                        nc.vector.tensor_scalar(
                            T2[:, :], a2, 4, None, ALU.logical_shift_right
                        )
                        nc.vector.tensor_scalar(
                            tm[:, :], a0, 0xFFF0, None, ALU.bitwise_and
                        )
                        _stt_u16(
                            nc, w3o[:, :, 0], T1[:, :], 8, tm[:, :],
                            ALU.logical_shift_right, ALU.bitwise_or,
                        )
                        nc.vector.tensor_scalar(
                            tm[:, :], T2[:, :], 4, None, ALU.logical_shift_right
                        )
                        _stt_u16(
                            nc, w3o[:, :, 1], T1[:, :], 8, tm[:, :],
                            ALU.logical_shift_left, ALU.bitwise_or,
                        )
                        nc.vector.tensor_scalar(
                            tm[:, :], a3, 4, None, ALU.logical_shift_right
                        )
                        _stt_u16(
                            nc, w3o[:, :, 2], T2[:, :], 12, tm[:, :],
                            ALU.logical_shift_left, ALU.bitwise_or,
                        )
                        zci = zc // nz
                        if last:
                            skip = XBO - lv.w_last
                            nc.sync.dma_start(
                                ys[R][(xo + skip) * C :, zci, :],
                                pw[skip * C :, :],
                            )
                        else:
                            nc.sync.dma_start(
                                ys[R][xo * C : xo * C + 96, zci, :],
                                pw[:, :],
                            )
    nc.finalize()
    return nc


def _build_wt(weight):
    # weight [Cout, Cin, kz, ky, kx]; WT[xi*16+ci, t, xo*16+co] = w[co,ci,a,b,xi-xo]
    w = np.asarray(weight, np.float32)
    WT = np.zeros((XBI, C, 9, XBO, C), np.float16)
    for t in range(9):
        a, b = t // 3, t % 3
        for xo_ in range(XBO):
            for d in range(3):
                WT[xo_ + d, :, t, xo_, :] = w[:, :, a, b, d].T
    return np.ascontiguousarray(WT.reshape(WT_ELEMS))


_ST = None  # lazy global state
_ST_LOCK = threading.Lock()


class _State:
    def __init__(self):
        import jax
        from concourse import bass2jax as b2j

        self.jax = jax
        nc = _build_nc()
        b2j.install_neuronx_cc_hook()
        # the kernel body never reads the partition id; bind it as 0
        part_name = (
            nc.partition_id_tensor.name if nc.partition_id_tensor is not None else None
        )
        in_names = ("xin", "yout") + ((part_name,) if part_name else ())

        out_aval = jax.core.ShapedArray((YN,), np.float16)

        def _body(xin_arr, ydummy):
            operands = [xin_arr, ydummy]
            if part_name is not None:
                operands.append(b2j.partition_id_tensor())
            outs = b2j._bass_exec_p.bind(
                *operands,
                out_avals=(out_aval,),
                in_names=in_names,
                out_names=("yout",),
                lowering_input_output_aliases=(),
                sim_require_finite=True,
                sim_require_nnan=True,
                nc=nc,
            )
            return outs[0]

        self.jfn = jax.jit(_body, keep_unused=True)
        self.devs = jax.devices()[:8]

        # persistent on-device stand-ins for the zero-init output operand
        # (every output element is DMA-written by the kernel, so their
        # contents never reach the result)
        mk = jax.jit(lambda a: jax.numpy.broadcast_to(a, (YN,)))
        self.dummies = []
        for d in self.devs:
            anchor = jax.device_put(np.zeros((), np.float16), d)
            self.dummies.append(jax.block_until_ready(mk(anchor)))

        # host buffers: fused per-core input rows + transposed fp16 scratch
        self.XG = np.zeros((8, XN), np.float16)
        self.S = {}
        self.VO = {}
        for core in range(8):
            for R in RESOLUTIONS:
                lv = LVLS[R]
                self.S[(core, R)] = np.zeros((R, C, lv.Zi, R), np.float16)
                self.VO[(core, R)] = np.empty((R * C, lv.Zo * R), np.uint16)
        self.fetch_pool = ThreadPoolExecutor(8)
        self.exec_pool = ThreadPoolExecutor(8)


def _get_state():
    global _ST
    if _ST is None:
        with _ST_LOCK:
            if _ST is None:
                _ST = _State()
    return _ST


def _pack_core(st, core, inp):
    bi, h = core // 2, core % 2
    row = st.XG[core]
    for R in RESOLUTIONS:
        lv = LVLS[R]
        x = inp[bi, LOFF[R] : LOFF[R] + R**3].reshape(R, R, R, C)
        zlo = h * lv.Zo - 1
        s0, s1 = max(zlo, 0), min(zlo + lv.Zi, R)
        S = st.S[(core, R)]
        # z-halo rows outside the grid are zero (S is zero-initialized and
        # the zero rows are per-core constant, but keep it explicit + cheap)
        np.copyto(S[:, :, s0 - zlo : s1 - zlo, :], x[s0:s1].transpose(2, 3, 0, 1))
        # pack fp16 -> 12 bit (round-to-nearest via +8 on the u16 view;
        # safe: no inf/nan and |x| << fp16 max). 4 values -> 3 words.
        A = S.reshape(R * C, lv.L).view(np.uint16) + np.uint16(8)
        a0, a1, a2, a3 = A[:, 0::4], A[:, 1::4], A[:, 2::4], A[:, 3::4]
        Wd = row[XOFF[R] : XOFF[R] + lv.in_elems].view(np.uint16).reshape(
            R * C, lv.G, 3
        )
        np.bitwise_or(a0 & np.uint16(0xFFF0), a1 >> 12, out=Wd[:, :, 0])
        np.bitwise_or((a1 << 4) & np.uint16(0xFF00), a2 >> 8, out=Wd[:, :, 1])
        np.bitwise_or((a2 << 8) & np.uint16(0xF000), a3 >> 4, out=Wd[:, :, 2])


def _unpack_core(st, core, ya, out):
    bi, h = core // 2, core % 2
    for R in RESOLUTIONS:
        lv = LVLS[R]
        rows = lv.Zo * R * R
        dst = out[
            bi, LOFF[R] + h * rows : LOFF[R] + (h + 1) * rows
        ].reshape(lv.Zo, R, R, C)
        # unpack 12-bit words -> fp16 bits (3 words -> 4 values)
        W3 = ya[YOFF[R] : YOFF[R] + lv.out_elems].view(np.uint16).reshape(
            R * C, lv.Zo * R // 4, 3
        )
        w0, w1, w2 = W3[:, :, 0], W3[:, :, 1], W3[:, :, 2]
        V = st.VO[(core, R)]
        np.bitwise_and(w0, np.uint16(0xFFF0), out=V[:, 0::4])
        np.bitwise_or(
            w0 << 12, (w1 >> 4) & np.uint16(0x0FF0), out=V[:, 1::4]
        )
        np.bitwise_or(
            w1 << 8, (w2 >> 8) & np.uint16(0x00F0), out=V[:, 2::4]
        )
        np.left_shift(w2, 4, out=V[:, 3::4])
        src = V.view(np.float16).reshape(R, C, lv.Zo, R)
        np.copyto(dst, src.transpose(2, 3, 0, 1))


def _run(inputs, trace=False):
    st = _get_state()
    jax = st.jax
    inp = np.asarray(inputs["input"], np.float32)
    weight = np.asarray(inputs["weight"], np.float32)
    bias = np.asarray(inputs["bias"], np.float32)

    wt_row = _build_wt(weight)
    b96 = np.tile(bias.astype(np.float16), XBO)
    for core in range(8):
        st.XG[core, 0:WT_ELEMS] = wt_row
        st.XG[core, WT_ELEMS : WT_ELEMS + BS_ELEMS] = b96

    out = np.empty((B, N_TOTAL, C), np.float32)

    def _exec_and_fetch(core, xdev):
        ydev = st.jfn(xdev, st.dummies[core])
        ya = np.asarray(ydev)
        _unpack_core(st, core, ya, out)

    futs = []
    for core in range(8):
        _pack_core(st, core, inp)
        xdev = jax.device_put(st.XG[core], st.devs[core])
        futs.append(st.exec_pool.submit(_exec_and_fetch, core, xdev))
    for f in futs:
        f.result()
    return out, None


def kernel(**inputs):
    out, _ = _run(inputs)
    return out


# revision 22
# speedup vs baseline: 1.5383x; 1.0370x over previous
"""Multi-resolution 3D conv (3x3x3, Cin=Cout=16) on 8 TRN2 NeuronCores.

Strategy:
- 8 cores = 4 batches x 2 z-halves. Each core runs an INDEPENDENT
  single-device bass exec (no shard_map barrier), so host->device upload,
  compute, and device->host download of different cores pipeline over the
  (slow, ~48MB/s shared) axon tunnel. The tunnel is the bottleneck, so
  everything is about minimizing bytes on the wire.
- All tunnel traffic is fp16 (tolerance 2e-2 >> fp16 error ~5e-4):
  one fused input tensor per core [weights | bias | 4 level slabs], one
  fused output tensor per core. No zero output buffers are uploaded:
  every output element is written by the kernel, so persistent on-device
  dummies stand in for the zero-init operands.
- No spatial zero-padding is uploaded either. Input slabs are exactly
  [R*16, Zi, R] (z-halo only). SAME-conv y-boundary handling is done with
  partial-width matmul taps; x-boundary handling with partition memsets
  on the first/last x-block tiles. Output is exactly [R*16, Zo, R].
- Per level (R in 16/32/48/64): 3x3x3 conv = 9 PSUM-accumulated matmuls,
  one per (kz, ky) tap. The kx 3-tap conv is folded into a block-Toeplitz
  stationary matrix: K = 8 input x-positions x 16 Cin = 128 partitions,
  M = 6 output x-positions x 16 Cout = 96. The last x-block is SHIFTED to
  x = R-6 (overlapping the previous block); only its non-overlapping
  output columns are written to DRAM.
"""

import os
import sys
import threading
from concurrent.futures import ThreadPoolExecutor

for _p in ("/opt/trn_rl_repo",):
    if os.path.isdir(_p) and _p not in sys.path:
        sys.path.insert(0, _p)

import numpy as np

import concourse.bacc as bacc
import concourse.mybir as mybir
from concourse.tile import TileContext

RESOLUTIONS = (16, 32, 48, 64)
B, C = 4, 16
N_TOTAL = sum(r**3 for r in RESOLUTIONS)
XBO, XBI = 6, 8  # output / input x-positions per x-block
F16 = mybir.dt.float16
F32 = mybir.dt.float32

# nz: output z-rows per matmul chunk (nz * R <= 512, one PSUM bank)
NZ = {64: 8, 48: 8, 32: 16, 16: 8}

WT_ELEMS = 128 * 9 * 96
BS_ELEMS = 96

# tap order: first tap is the full-width (b=1) one so its start=True
# write covers every PSUM element of the chunk
TAPS = [(0, 1), (0, 0), (0, 2), (1, 1), (1, 0), (1, 2), (2, 1), (2, 0), (2, 2)]


class _Lvl:
    def __init__(self, R):
        self.R = R
        self.Xp = -(-R // XBO)          # number of x-blocks
        self.Zo = R // 2                # output z-rows per core
        self.Zi = self.Zo + 2           # z rows incl halo
        self.L = self.Zi * R            # free elems per partition
        self.G = self.L // 4            # 12-bit pack groups per partition
        self.W = 3 * self.G             # packed u16 words per partition
        self.in_elems = R * C * self.W         # packed [R*16, W]
        nz = NZ[R]
        self.L2 = nz * R                # out elems per partition per chunk
        self.G2 = self.L2 // 4
        self.W2 = 3 * self.G2
        self.nch = self.Zo // nz
        self.out_elems = R * C * self.nch * self.W2  # packed [R*16, nch, W2]
        self.w_last = R - (self.Xp - 1) * XBO  # fresh x-cols in last block

    def xo(self, xb):
        return min(xb * XBO, self.R - XBO)


LVLS = {R: _Lvl(R) for R in RESOLUTIONS}
ORDER = (64, 48, 32, 16)  # big level first keeps PE warm

# fused input layout per core: [wt | bias | lvl64 | lvl48 | lvl32 | lvl16]
XOFF = {}
_o = WT_ELEMS + BS_ELEMS
for _R in ORDER:
    XOFF[_R] = _o
    _o += LVLS[_R].in_elems
XN = _o
YOFF = {}
_o = 0
for _R in ORDER:
    YOFF[_R] = _o
    _o += LVLS[_R].out_elems
YN = _o

LOFF = {}  # level offset in the flat [B, N_TOTAL, C] input
_o = 0
for _R in RESOLUTIONS:
    LOFF[_R] = _o
    _o += _R**3


def _stt_u16(nc, out, in0, imm, in1, op0, op1):
    """scalar_tensor_tensor with a uint16-typed immediate:
    out = (in0 op0 imm) op1 in1. The stock helper types immediates as
    float32, which the BIR verifier rejects for bitvec ops on u16."""
    eng = nc.vector
    return eng.add_instruction(
        mybir.InstTensorScalarPtr(
            name=eng.bass.get_next_instruction_name(),
            is_scalar_tensor_tensor=True,
            op0=op0,
            op1=op1,
            ins=[
                eng.lower_ap(in0),
                mybir.ImmediateValue(dtype=mybir.dt.uint16, value=imm),
                eng.lower_ap(in1),
            ],
            outs=[eng.lower_ap(out)],
        )
    )


def _build_nc():
    nc = bacc.Bacc(target_bir_lowering=False)
    xin = nc.dram_tensor("xin", [XN], F16, kind="ExternalInput")
    yout = nc.dram_tensor("yout", [YN], F16, kind="ExternalOutput")

    wt = xin[0:WT_ELEMS].rearrange("(k t m) -> k t m", t=9, m=96)
    bs = xin[WT_ELEMS : WT_ELEMS + BS_ELEMS].rearrange("(p o) -> p o", o=1)
    xs, ys = {}, {}
    for R in ORDER:
        lv = LVLS[R]
        # packed 12-bit input: [R*16 partitions, W u16 words]
        xs[R] = xin[XOFF[R] : XOFF[R] + lv.in_elems].rearrange(
            "(p w) -> p w", w=lv.W
        )
        # packed 12-bit output: [R*16 partitions, z-chunks, W2 words]
        ys[R] = yout[YOFF[R] : YOFF[R] + lv.out_elems].rearrange(
            "(p c w) -> p c w", c=lv.nch, w=lv.W2
        )

    U16 = mybir.dt.uint16
    ALU = mybir.AluOpType

    with TileContext(nc) as tc:
        with (
            tc.tile_pool(name="wp", bufs=1) as wp,
            tc.tile_pool(name="qp", bufs=4) as qp,
            tc.tile_pool(name="tp", bufs=4) as tp,
            tc.tile_pool(name="ip", bufs=4) as ip,
            tc.tile_pool(name="op", bufs=6) as op,
            tc.tile_pool(name="pp", bufs=4, space="PSUM") as pp,
            tc.tile_pool(name="dp", bufs=1, space="PSUM") as dp,
        ):
            wt_sb = wp.tile([128, 9, 96], F16, name="wt_sb")
            nc.sync.dma_start(wt_sb[:, :, :], wt)
            bs_sb = wp.tile([96, 1], F16, name="bs_sb")
            nc.sync.dma_start(bs_sb[:, :], bs)
            # The S3 LDWEIGHTS slot only fits one semaphore wait, so make
            # sure every real matmul needs at most one: absorb each DMA's
            # completion wait with a tiny throwaway PE/DVE op first.
            dps = dp.tile([1, 2], F32, name="dps")
            scr = wp.tile([96, 1], F32, name="scr")
            nc.tensor.matmul(
                dps[0:1, 0:1], wt_sb[:, 0, 0:1], wt_sb[:, 0, 0:1],
                start=True, stop=True,
            )
            # scr doubles as the fp32 bias used by every tensor_scalar_add
            nc.vector.tensor_copy(scr[:, :], bs_sb[:, :])

            for R in ORDER:
                lv = LVLS[R]
                nz = NZ[R]
                for xb in range(lv.Xp):
                    xo = lv.xo(xb)
                    first = xb == 0
                    last = xb == lv.Xp - 1
                    # packed 12-bit words for this block's 128 partitions
                    pt = qp.tile([128, lv.W], F16, tag="pt", name=f"pt{R}_{xb}")
                    if first:
                        # x = -1 does not exist: zero words unpack to zero.
                        # memset a 32-aligned span; DMA overwrites 16:32.
                        nc.vector.memset(pt[0:32, :], 0.0)
                        nc.sync.dma_start(pt[16:128, :], xs[R][0:112, :])
                    elif last:
                        nc.vector.memset(pt[96:128, :], 0.0)
                        nc.sync.dma_start(
                            pt[0:112, :], xs[R][(lv.R - 7) * C :, :]
                        )
                    else:
                        nc.sync.dma_start(
                            pt[:, :],
                            xs[R][(xo - 1) * C : (xo - 1) * C + 128, :],
                        )
                    it = ip.tile(
                        [128, lv.Zi, lv.R], F16, tag="it", name=f"it{R}_{xb}"
                    )
                    # DVE unpack: 3 words -> 4 fp16 values per group
                    w3 = pt[:, :].bitcast(U16).rearrange(
                        "p (g t) -> p g t", t=3
                    )
                    o4 = it[:, :, :].bitcast(U16).rearrange(
                        "p z y -> p (z y)"
                    ).rearrange("p (g f) -> p g f", f=4)
                    w0, w1, w2 = w3[:, :, 0], w3[:, :, 1], w3[:, :, 2]
                    tb = tp.tile([128, lv.G], U16, tag="tb", name=f"tb{R}_{xb}")
                    nc.vector.tensor_scalar(
                        o4[:, :, 0], w0, 0xFFF0, None, ALU.bitwise_and
                    )
                    nc.vector.tensor_scalar(
                        tb[:, :], w1, 4, 0x0FF0,
                        ALU.logical_shift_right, ALU.bitwise_and,
                    )
                    _stt_u16(
                        nc, o4[:, :, 1], w0, 12, tb[:, :],
                        ALU.logical_shift_left, ALU.bitwise_or,
                    )
                    nc.vector.tensor_scalar(
                        tb[:, :], w2, 8, 0x00F0,
                        ALU.logical_shift_right, ALU.bitwise_and,
                    )
                    _stt_u16(
                        nc, o4[:, :, 2], w1, 8, tb[:, :],
                        ALU.logical_shift_left, ALU.bitwise_or,
                    )
                    nc.vector.tensor_scalar(
                        o4[:, :, 3], w2, 4, None, ALU.logical_shift_left
                    )
                    nc.tensor.matmul(
                        dps[0:1, 0:1], it[:, 0, 0:1], it[:, 0, 0:1],
                        start=True, stop=True,
                    )
                    for zc in range(0, lv.Zo, nz):
                        ps = pp.tile([96, nz, R], F32, tag="ps", name=f"ps{R}_{xb}_{zc}")
                        for ti, (a, b) in enumerate(TAPS):
                            # SAME-conv y boundary: tap b contributes to
                            # out y in [max(0,1-b), R-b+1) cap [0, R)
                            ylo, yhi = (1, R) if b == 0 else (0, R - 1) if b == 2 else (0, R)
                            nc.tensor.matmul(
                                ps[:, :, ylo:yhi],
                                wt_sb[:, a * 3 + b, :],
                                it[:, zc + a : zc + a + nz, ylo + b - 1 : yhi + b - 1],
                                start=(ti == 0),
                                stop=(ti == 8),
                            )
                        ot = op.tile([96, nz, R], F16, tag="ot", name=f"ot{R}_{xb}_{zc}")
                        nc.vector.tensor_scalar_add(ot[:, :, :], ps[:, :, :], scr[:, :])
                        # DVE pack fp16 -> 12 bit (round-to-nearest via +8;
                        # the add must be its own instr: no arith+bitwise mix)
                        au = tp.tile([96, lv.L2], U16, tag="au", name=f"au{R}_{xb}_{zc}")
                        nc.vector.tensor_scalar(
                            au[:, :],
                            ot[:, :, :].bitcast(U16).rearrange("p z y -> p (z y)"),
                            8, None, ALU.add,
                        )
                        a4 = au[:, :].rearrange("p (g f) -> p g f", f=4)
                        a0, a1, a2, a3 = (a4[:, :, j] for j in range(4))
                        pw = op.tile([96, lv.W2], F16, tag="pw", name=f"pw{R}_{xb}_{zc}")
                        w3o = pw[:, :].bitcast(U16).rearrange(
                            "p (g t) -> p g t", t=3
                        )
                        T1 = tp.tile([96, lv.G2], U16, tag="T1", name=f"T1{R}_{xb}_{zc}")
                        T2 = tp.tile([96, lv.G2], U16, tag="T2", name=f"T2{R}_{xb}_{zc}")
                        tm = tp.tile([96, lv.G2], U16, tag="tm", name=f"tm{R}_{xb}_{zc}")
                        nc.vector.tensor_scalar(
                            T1[:, :], a1, 4, None, ALU.logical_shift_right
                        )
                        nc.vector.tensor_scalar(
                            T2[:, :], a2, 4, None, ALU.logical_shift_right
                        )
                        nc.vector.tensor_scalar(
                            tm[:, :], a0, 0xFFF0, None, ALU.bitwise_and
                        )
                        _stt_u16(
                            nc, w3o[:, :, 0], T1[:, :], 8, tm[:, :],
                            ALU.logical_shift_right, ALU.bitwise_or,
                        )
                        nc.vector.tensor_scalar(
                            tm[:, :], T2[:, :], 4, None, ALU.logical_shift_right
                        )
                        _stt_u16(
                            nc, w3o[:, :, 1], T1[:, :], 8, tm[:, :],
                            ALU.logical_shift_left, ALU.bitwise_or,
                        )
                        nc.vector.tensor_scalar(
                            tm[:, :], a3, 4, None, ALU.logical_shift_right
                        )
                        _stt_u16(
                            nc, w3o[:, :, 2], T2[:, :], 12, tm[:, :],
                            ALU.logical_shift_left, ALU.bitwise_or,
                        )
                        zci = zc // nz
                        if last:
                            skip = XBO - lv.w_last
                            nc.sync.dma_start(
                                ys[R][(xo + skip) * C :, zci, :],
                                pw[skip * C :, :],
                            )
                        else:
                            nc.sync.dma_start(
                                ys[R][xo * C : xo * C + 96, zci, :],
                                pw[:, :],
                            )
    nc.finalize()
    return nc


def _build_wt(weight):
    # weight [Cout, Cin, kz, ky, kx]; WT[xi*16+ci, t, xo*16+co] = w[co,ci,a,b,xi-xo]
    w = np.asarray(weight, np.float32)
    WT = np.zeros((XBI, C, 9, XBO, C), np.float16)
    for t in range(9):
        a, b = t // 3, t % 3
        for xo_ in range(XBO):
            for d in range(3):
                WT[xo_ + d, :, t, xo_, :] = w[:, :, a, b, d].T
    return np.ascontiguousarray(WT.reshape(WT_ELEMS))


_ST = None  # lazy global state
_ST_LOCK = threading.Lock()


class _State:
    def __init__(self):
        import jax
        from concourse import bass2jax as b2j

        self.jax = jax
        nc = _build_nc()
        b2j.install_neuronx_cc_hook()
        # the kernel body never reads the partition id; bind it as 0
        part_name = (
            nc.partition_id_tensor.name if nc.partition_id_tensor is not None else None
        )
        in_names = ("xin", "yout") + ((part_name,) if part_name else ())

        out_aval = jax.core.ShapedArray((YN,), np.float16)

        def _body(xin_arr, ydummy):
            operands = [xin_arr, ydummy]
            if part_name is not None:
                operands.append(b2j.partition_id_tensor())
            outs = b2j._bass_exec_p.bind(
                *operands,
                out_avals=(out_aval,),
                in_names=in_names,
                out_names=("yout",),
                lowering_input_output_aliases=(),
                sim_require_finite=True,
                sim_require_nnan=True,
                nc=nc,
            )
            return outs[0]

        self.jfn = jax.jit(_body, keep_unused=True)
        self.devs = jax.devices()[:8]

        # persistent on-device stand-ins for the zero-init output operand
        # (every output element is DMA-written by the kernel, so their
        # contents never reach the result)
        mk = jax.jit(lambda a: jax.numpy.broadcast_to(a, (YN,)))
        self.dummies = []
        for d in self.devs:
            anchor = jax.device_put(np.zeros((), np.float16), d)
            self.dummies.append(jax.block_until_ready(mk(anchor)))

        # host buffers: fused per-core input rows + transposed fp16 scratch
        self.XG = np.zeros((8, XN), np.float16)
        self.S = {}
        self.VO = {}
        for core in range(8):
            for R in RESOLUTIONS:
                lv = LVLS[R]
                self.S[(core, R)] = np.zeros((R, C, lv.Zi, R), np.float16)
                self.VO[(core, R)] = np.empty((R * C, lv.Zo * R), np.uint16)
        self.exec_pool = ThreadPoolExecutor(8)


def _get_state():
    global _ST
    if _ST is None:
        with _ST_LOCK:
            if _ST is None:
                _ST = _State()
    return _ST


def _pack_core(st, core, inp):
    bi, h = core // 2, core % 2
    row = st.XG[core]
    for R in RESOLUTIONS:
        lv = LVLS[R]
        x = inp[bi, LOFF[R] : LOFF[R] + R**3].reshape(R, R, R, C)
        zlo = h * lv.Zo - 1
        s0, s1 = max(zlo, 0), min(zlo + lv.Zi, R)
        S = st.S[(core, R)]
        # z-halo rows outside the grid are zero (S is zero-initialized and
        # the zero rows are per-core constant, but keep it explicit + cheap)
        np.copyto(S[:, :, s0 - zlo : s1 - zlo, :], x[s0:s1].transpose(2, 3, 0, 1))
        # pack fp16 -> 12 bit (round-to-nearest via +8 on the u16 view;
        # safe: no inf/nan and |x| << fp16 max). 4 values -> 3 words.
        A = S.reshape(R * C, lv.L).view(np.uint16) + np.uint16(8)
        a0, a1, a2, a3 = A[:, 0::4], A[:, 1::4], A[:, 2::4], A[:, 3::4]
        Wd = row[XOFF[R] : XOFF[R] + lv.in_elems].view(np.uint16).reshape(
            R * C, lv.G, 3
        )
        np.bitwise_or(a0 & np.uint16(0xFFF0), a1 >> 12, out=Wd[:, :, 0])
        np.bitwise_or((a1 << 4) & np.uint16(0xFF00), a2 >> 8, out=Wd[:, :, 1])
        np.bitwise_or((a2 << 8) & np.uint16(0xF000), a3 >> 4, out=Wd[:, :, 2])


def _unpack_core(st, core, ya, out):
    bi, h = core // 2, core % 2
    for R in RESOLUTIONS:
        lv = LVLS[R]
        rows = lv.Zo * R * R
        dst = out[
            bi, LOFF[R] + h * rows : LOFF[R] + (h + 1) * rows
        ].reshape(lv.Zo, R, R, C)
        # unpack 12-bit words -> fp16 bits (3 words -> 4 values)
        W3 = ya[YOFF[R] : YOFF[R] + lv.out_elems].view(np.uint16).reshape(
            R * C, lv.Zo * R // 4, 3
        )
        w0, w1, w2 = W3[:, :, 0], W3[:, :, 1], W3[:, :, 2]
        V = st.VO[(core, R)]
        np.bitwise_and(w0, np.uint16(0xFFF0), out=V[:, 0::4])
        np.bitwise_or(
            w0 << 12, (w1 >> 4) & np.uint16(0x0FF0), out=V[:, 1::4]
        )
        np.bitwise_or(
            w1 << 8, (w2 >> 8) & np.uint16(0x00F0), out=V[:, 2::4]
        )
        np.left_shift(w2, 4, out=V[:, 3::4])
        src = V.view(np.float16).reshape(R, C, lv.Zo, R)
        np.copyto(dst, src.transpose(2, 3, 0, 1))


def _run(inputs, trace=False):
    st = _get_state()
    jax = st.jax
    inp = np.asarray(inputs["input"], np.float32)
    weight = np.asarray(inputs["weight"], np.float32)
    bias = np.asarray(inputs["bias"], np.float32)

    wt_row = _build_wt(weight)
    b96 = np.tile(bias.astype(np.float16), XBO)
    for core in range(8):
        st.XG[core, 0:WT_ELEMS] = wt_row
        st.XG[core, WT_ELEMS : WT_ELEMS + BS_ELEMS] = b96

    out = np.empty((B, N_TOTAL, C), np.float32)

    def _exec_and_fetch(core, xdev):
        ydev = st.jfn(xdev, st.dummies[core])
        ya = np.asarray(ydev)
        _unpack_core(st, core, ya, out)

    futs = []
    for core in range(8):
        _pack_core(st, core, inp)
        xdev = jax.device_put(st.XG[core], st.devs[core])
        futs.append(st.exec_pool.submit(_exec_and_fetch, core, xdev))
    for f in futs:
        f.result()
    return out, None


def kernel(**inputs):
    out, _ = _run(inputs)
    return out
